# revision 34
# baseline (speedup 1.0000x reference)
"""Trainium2 Bass kernel for CrossModalAttention2d (pipelined rewrite).

Reference computation (per batch element b):
    q = Wq @ face[b] + bq          # [64, 1024]   (face as [C=512, N=1024])
    k = Wk @ audio[b] + bk         # [64, 1024]
    v = Wv @ audio[b] + bv         # [512, 1024]
    attn = softmax(q^T k / 8, axis=-1)          # [1024, 1024]
    out = gamma * (v @ attn^T) + face[b]        # [512, 1024]

Distribution: data-parallel over batch B=32 across 8 NeuronCores
(4 batch elements per core); every core holds the full (small) weights.

Design notes (v2 — software-pipelined, engine-balanced):
- 2-stage pipeline per core: iter n runs {proj q/k/v, energy, exp} for
  batch n on TensorE/ACT while {PV, softmax-normalize, residual, out-DMA}
  for batch n-1 run concurrently.  Tensor issue order interleaves the two
  so the PE never drains a single dependency chain.
- Energy is computed transposed ET[nk,nq] = k^T q with bf16 operands and
  K=128 (q/k host-duplicated into both partition halves => ET = 2E).
- PSUM->SBUF traffic is the scarce resource (GPSIMD has no PSUM access on
  TRN2, so only ACT+DVE can drain PSUM): every such op is [128,1024]
  (PSUM pairs merged) to amortize the ~200-370ns fixed access latency.
  GPSIMD gets the one SBUF-only bulk job: the bf16 residual adds.
- Softmax denominators come from a 5th PV chunk whose stationary is an
  all-ones fp8 tile: S lands broadcast across all 128 partitions in the
  same PSUM rotation the PV chunks use (no separate sum pass).
- PSUM budget = exactly 8 banks: tag "ep" 2x[128,1024] (energy groups) +
  tag "u" 2x[128,2,512] rotating (proj / vproj / PV / sum pairs).
- All heavy matmuls run fp8 DoubleRow (0.5 cyc/row): projections, vproj,
  PV.  Energy is bf16 (the K=64 contraction can't fill 256 rows).
- Weights are host-scaled x16 to move their ~0.02-magnitude entries out
  of fp8-denormal territory; the 1/16 is folded into the cast scales /
  the PV normalize multiply.  gamma and gamma*bv are folded on host into
  Wv and the bf16 residual respectively (exact: attn rows sum to 1).
- Residual path is bf16 end-to-end: face bf16 in, out bf16 back to HBM,
  upcast to fp32 on host (|err| ~ 2^-9 << 2e-2 gate).
- A dozen warm-up matmuls on constant tiles run during the initial DMAs
  so the PE p-state ramp (0.65/1.2 GHz -> 2.4 GHz after ~3us busy) is
  paid before real work; a dummy Exp preloads the ACT table.
"""

from contextlib import ExitStack

import ml_dtypes
import numpy as np

import concourse.bass as bass
import concourse.mybir as mybir
import concourse.tile as tile
from concourse import bacc
from concourse.bass import ds
from concourse.bass_utils import run_bass_kernel_spmd

N_CORES = 8
B = 32
C = 512
CQK = 64
N = 1024          # Nq = Nk = 32*32
H = W = 32
BPC = B // N_CORES  # batches per core
CC = C // 128       # 4 c-chunks
NT = N // 128       # 8 nk-tiles

BF16 = mybir.dt.bfloat16
FP8 = mybir.dt.float8e4
F32 = mybir.dt.float32
DR = mybir.MatmulPerfMode.DoubleRow

WS = 16.0           # host-side weight scale (fp8 denormal avoidance)
IWS = 1.0 / WS      # folded back on device
# energy PSUM = 2*E (q/k partition-duplicated); softmax wants exp(E/8)
EXP_SCALE = 1.0 / 16.0
# packed weights, bytes per partition: bq(4) bk(4) wq(512) wk(512) wv(2048)
WPACK_B = 3080

_PROGRAM = None


def _emit(nc, tc, ctx, io):
    face8, audio8, faceb, wpack, out = io
    AF = mybir.ActivationFunctionType
    ALU = mybir.AluOpType

    wpool = ctx.enter_context(tc.tile_pool(name="w", bufs=1))
    sb = ctx.enter_context(tc.tile_pool(name="sb", bufs=2))
    pp = ctx.enter_context(tc.tile_pool(name="pp", bufs=1, space="PSUM"))

    # --- constants + PE warm-up FIRST, with memsets on GPSIMD (its
    # queue is free earliest), so the HAM clock gate is released while
    # the first input DMAs are still in flight.  Memset through a u32
    # bitcast (4x fewer columns: fp8 memsets run at 1 byte/lane/cycle) ---
    ones = wpool.tile([128, 2, 128], FP8, tag="ones")
    nc.gpsimd.memset(ones[:].bitcast(mybir.dt.uint32), 0x38383838)  # fp8 1.0
    # preload the ACT exp table while DMAs run
    dum_i = wpool.tile([128, 1], F32, tag="dumi")
    nc.gpsimd.memset(dum_i[:], 0.0)
    dum_o = wpool.tile([128, 1], F32, tag="dumo")
    nc.scalar.activation(dum_o[:], dum_i[:], AF.Exp)

    # ~26 x 128-col warm matmuls on `ones` (~3.4us of cold matmuls = one
    # HAM window releases the clock gate); same stationary throughout so
    # LDWEIGHTS runs once
    for i in range(13):
        wp = pp.tile([128, 2, 512], F32, tag="u", bufs=2, name=f"warm{i}")
        nc.tensor.matmul(wp[:, 0, 0:128], ones[:], ones[:], start=True,
                         stop=True, perf_mode=DR)
        nc.tensor.matmul(wp[:, 1, 0:128], ones[:], ones[:], start=True,
                         stop=True, perf_mode=DR)

    # --- all weights in ONE packed DMA (per-DMA queue overhead is
    # ~600ns; five separate weight DMAs would push wv past vproj(0)) ---
    wp_sb = wpool.tile([128, WPACK_B], mybir.dt.uint8, tag="wpack")
    nc.sync.dma_start(wp_sb[:], wpack[:])
    bq_sb = wp_sb[:, 0:4].bitcast(F32)
    bk_sb = wp_sb[:, 4:8].bitcast(F32)
    wq_sb = wp_sb[:, 8:520].bitcast(FP8).rearrange("p (c m) -> p c m", c=CC)
    wk_sb = wp_sb[:, 520:1032].bitcast(FP8).rearrange("p (c m) -> p c m", c=CC)
    wv_sb = wp_sb[:, 1032:3080].bitcast(FP8).rearrange("p (c m) -> p c m", c=CC)

    # --- input DMA helper ---
    ins = {}

    def dma_in(b, split=False):
        f8 = sb.tile([128, CC, N], FP8, tag="face8", name=f"f8_{b}")
        a8 = sb.tile([128, CC, N], FP8, tag="aud8", name=f"a8_{b}")
        fb = sb.tile([128, CC, N], BF16, tag="faceb", bufs=3, name=f"fb_{b}")
        if split:
            # batch 0 rides the startup critical path: split the loads by
            # cc-halves so the first K=256 projection pass starts sooner
            for h in (0, 2):
                nc.sync.dma_start(f8[:, h:h + 2, :],
                                  face8[b, h:h + 2].transpose([1, 0, 2]))
                nc.sync.dma_start(a8[:, h:h + 2, :],
                                  audio8[b, h:h + 2].transpose([1, 0, 2]))
        else:
            nc.sync.dma_start(f8[:], face8[b].transpose([1, 0, 2]))
            nc.sync.dma_start(a8[:], audio8[b].transpose([1, 0, 2]))
        nc.sync.dma_start(fb[:], faceb[b].transpose([1, 0, 2]))
        ins[b] = (f8, a8, fb)

    dma_in(0, split=True)

    prev = None  # state of batch n-1: (b, pt, vt, recip, fb, outb)

    for n in range(BPC + 1):
        if n + 1 < BPC:
            dma_in(n + 1)

        have_f = n < BPC
        have_p = prev is not None

        if have_f:
            f8, a8, fb = ins.pop(n)
            # --- q/k projections (fp8 DoubleRow, K=512 as 2x256) ---
            qp = pp.tile([128, 2, 512], F32, tag="u", bufs=2, name=f"qp{n}")
            for kk in (0, 2):
                for j in range(2):
                    nc.tensor.matmul(qp[:, j, :], wq_sb[:, kk:kk + 2, :],
                                     f8[:, kk:kk + 2, ds(j * 512, 512)],
                                     start=(kk == 0), stop=(kk == 2),
                                     perf_mode=DR)
            q_sb = sb.tile([128, N], BF16, tag="q", name=f"q{n}")
            nc.scalar.activation(q_sb[:], qp[:], AF.Identity,
                                 bias=bq_sb[:], scale=IWS)

            kp = pp.tile([128, 2, 512], F32, tag="u", bufs=2, name=f"kp{n}")
            for kk in (0, 2):
                for j in range(2):
                    nc.tensor.matmul(kp[:, j, :], wk_sb[:, kk:kk + 2, :],
                                     a8[:, kk:kk + 2, ds(j * 512, 512)],
                                     start=(kk == 0), stop=(kk == 2),
                                     perf_mode=DR)
            k_sb = sb.tile([128, N], BF16, tag="k", name=f"k{n}")
            # split j-halves so energy group 0 unblocks after the first
            for j in range(2):
                nc.vector.tensor_scalar(k_sb[:, ds(j * 512, 512)],
                                        kp[:, j, :], IWS, bk_sb[:],
                                        ALU.mult, ALU.add)

            pt_sb = sb.tile([128, NT, N], FP8, tag="pt", name=f"pt{n}")
            vt_sb = sb.tile([128, NT, C], FP8, tag="vt", name=f"vt{n}")

        # --- PV piece emitters (interleaved into g-slots). `st` is the
        # per-batch PV state: [bp, pt, vt, fb, outb, op_s, recip] ---
        def piece_ops(st, tts):
            bp, pt_p, vt_p, fb_p, outb_p = st[:5]
            if st[5] is None:
                st[5] = pp.tile([128, 2, 512], F32, tag="u", bufs=2,
                                name=f"ops{n}_{bp}")
            op_s = st[5]
            for tt in tts:
                for j in range(2):
                    nc.tensor.matmul(op_s[:, j, :], ones[:],
                                     pt_p[:, tt:tt + 2, ds(j * 512, 512)],
                                     start=(tt == 0), stop=(tt == 6),
                                     perf_mode=DR)

        def piece_recip(st):
            recip = sb.tile([128, N], F32, tag="recip", name=f"rc{n}_{st[0]}")
            nc.vector.reciprocal_approx_fast(recip[:], st[5][:])
            st[6] = recip

        def piece_cc(st, cc, drain=False):
            bp, pt_p, vt_p, fb_p, outb_p = st[:5]
            recip = st[6]
            op = pp.tile([128, 2, 512], F32, tag="u", bufs=2,
                         name=f"op{n}_{bp}_{cc}")
            for tt in (0, 2, 4, 6):
                for j in range(2):
                    nc.tensor.matmul(op[:, j, :],
                                     vt_p[:, tt:tt + 2, ds(cc * 128, 128)],
                                     pt_p[:, tt:tt + 2, ds(j * 512, 512)],
                                     start=(tt == 0), stop=(tt == 6),
                                     perf_mode=DR)
            # out_attn = (op * 1/16) * recip   (wv was x16 on host)
            tmp = sb.tile([128, N], BF16, tag="tmp", bufs=3,
                          name=f"tmp{n}_{bp}_{cc}")
            if not drain:
                # steady state: one 1024-col normalize on DVE, add on
                # GPSIMD (SBUF-only bf16)
                nc.vector.scalar_tensor_tensor(tmp[:], op[:], IWS, recip[:],
                                               ALU.mult, ALU.mult)
                nc.gpsimd.tensor_add(outb_p[:, cc, :], tmp[:], fb_p[:, cc, :])
            else:
                # drain iteration: j-split chain fully on DVE so the tail
                # after the last matmul is short (gpsimd adds are 7x slower)
                for j in range(2):
                    js = ds(j * 512, 512)
                    nc.vector.scalar_tensor_tensor(
                        tmp[:, js], op[:, j, :], IWS, recip[:, js],
                        ALU.mult, ALU.mult)
                    nc.vector.tensor_add(outb_p[:, cc, js],
                                         tmp[:, js], fb_p[:, cc, js])
            nc.sync.dma_start(out[bp, cc], outb_p[:, cc, :])

        if have_p and have_f:
            st_p = prev
            if n == 1:
                # fill iteration: exp(0) is still draining on ACT — place
                # PV(0) in the back half so the in-order tensor queue
                # never blocks on a not-yet-exp'd PT tile
                slot_plan = {4: [lambda: piece_ops(st_p, (0, 2))],
                             5: [lambda: piece_ops(st_p, (4, 6)),
                                 lambda: piece_recip(st_p)],
                             6: [lambda: piece_cc(st_p, 0),
                                 lambda: piece_cc(st_p, 1)],
                             7: [lambda: piece_cc(st_p, 2),
                                 lambda: piece_cc(st_p, 3)]}
            else:
                slot_plan = {0: [lambda: piece_ops(st_p, (0, 2))],
                             1: [lambda: piece_ops(st_p, (4, 6)),
                                 lambda: piece_recip(st_p)],
                             2: [lambda: piece_cc(st_p, 0)],
                             3: [lambda: piece_cc(st_p, 1)],
                             4: [lambda: piece_cc(st_p, 2)],
                             5: [lambda: piece_cc(st_p, 3)]}
        else:
            slot_plan = {}

        # --- interleaved: energy+exp+vproj (batch n) / PV chunks (n-1) ---
        for g in range(NT):
            if have_f:
                # energy group g: ET[g-tile, :] = k^T q  (bf16, K=128 dup)
                ep = pp.tile([128, 1024], F32, tag="ep", bufs=2,
                             name=f"ep{n}_{g}")
                for j in range(2):
                    nc.tensor.matmul(ep[:, ds(j * 512, 512)],
                                     k_sb[:, ds(g * 128, 128)],
                                     q_sb[:, ds(j * 512, 512)],
                                     start=True, stop=True)
                nc.scalar.activation(pt_sb[:, g, :], ep[:], AF.Exp,
                                     scale=EXP_SCALE)
                # vproj pair: Vt[2 nk-tiles, c] = (16*gamma*Wv) @ audio
                if g % 2 == 0:
                    vp = pp.tile([128, 2, 512], F32, tag="u", bufs=2,
                                 name=f"vp{n}_{g}")
                for kk in (0, 2):
                    nc.tensor.matmul(vp[:, g % 2, :],
                                     a8[:, kk:kk + 2, ds(g * 128, 128)],
                                     wv_sb[:, kk:kk + 2, :],
                                     start=(kk == 0), stop=(kk == 2),
                                     perf_mode=DR)
                if g % 2 == 1:
                    # vt cast: 1 of 4 on ACT (its only slack), rest on DVE
                    if g == 1:
                        nc.scalar.activation(vt_sb[:, g - 1:g + 1, :], vp[:],
                                             AF.Copy)
                    else:
                        nc.vector.tensor_copy(vt_sb[:, g - 1:g + 1, :], vp[:])
            for p in slot_plan.get(g, []):
                p()

        if have_p and not have_f:
            # drain iteration: no forward work to interleave with
            piece_ops(prev, (0, 2))
            piece_ops(prev, (4, 6))
            piece_recip(prev)
            for cc in range(CC):
                piece_cc(prev, cc, drain=True)

        if have_f:
            outb = sb.tile([128, CC, N], BF16, tag="outb", name=f"ob{n}")
            prev = [n, pt_sb, vt_sb, fb, outb, None, None]
        else:
            prev = None


def _build_program():
    global _PROGRAM
    if _PROGRAM is not None:
        return _PROGRAM
    nc = bacc.Bacc("TRN2", target_bir_lowering=False, debug=False,
                   num_devices=N_CORES)
    d = {}
    d["face8"] = nc.dram_tensor("face8", [BPC, CC, 128, N], FP8,
                                kind="ExternalInput").ap()
    d["audio8"] = nc.dram_tensor("audio8", [BPC, CC, 128, N], FP8,
                                 kind="ExternalInput").ap()
    d["faceb"] = nc.dram_tensor("faceb", [BPC, CC, 128, N], BF16,
                                kind="ExternalInput").ap()
    d["wpack"] = nc.dram_tensor("wpack", [128, WPACK_B], mybir.dt.uint8,
                                kind="ExternalInput").ap()
    d["out"] = nc.dram_tensor("out", [BPC, CC, 128, N], BF16,
                              kind="ExternalOutput").ap()

    io = (d["face8"], d["audio8"], d["faceb"], d["wpack"], d["out"])
    with tile.TileContext(nc) as tc:
        with ExitStack() as ctx:
            _emit(nc, tc, ctx, io)
    nc.compile()
    _PROGRAM = nc
    return nc


def _make_in_maps(face_feat, audio_feat, Wq, bq, Wk, bk, Wv, bv, gamma):
    bf16 = ml_dtypes.bfloat16
    fp8 = ml_dtypes.float8_e4m3fn
    face = np.ascontiguousarray(face_feat.reshape(B, C, N), dtype=np.float32)
    audio = np.ascontiguousarray(audio_feat.reshape(B, C, N), dtype=np.float32)
    g = np.float32(np.asarray(gamma).reshape(-1)[0])

    # residual folds in gamma*bv (v-bias passes through softmax exactly)
    faceb = (face + (g * bv.astype(np.float32))[None, :, None])
    faceb = faceb.astype(bf16).reshape(B, CC, 128, N)

    face8 = face.astype(fp8).reshape(B, CC, 128, N)
    audio8 = audio.astype(fp8).reshape(B, CC, 128, N)

    def chunk_t(wT):  # [C, M] -> [128, CC, M]
        return np.ascontiguousarray(
            wT.reshape(CC, 128, -1).transpose(1, 0, 2))

    # q/k weights duplicated along M so the projections emit q/k into both
    # partition halves (energy contracts K=128 = 2x the true 64).
    # All weights are scaled x16 on host to stay clear of fp8 denormals;
    # the device folds the 1/16 back in (cast scales / normalize mult).
    wqT = chunk_t((np.concatenate([Wq.T, Wq.T], axis=1)
                   .astype(np.float32) * WS).astype(fp8))
    wkT = chunk_t((np.concatenate([Wk.T, Wk.T], axis=1)
                   .astype(np.float32) * WS).astype(fp8))
    wvT = chunk_t((Wv.astype(np.float32).T * (WS * g)).astype(fp8))
    bq2 = np.tile(bq.astype(np.float32).reshape(CQK, 1), (2, 1))
    bk2 = np.tile(bk.astype(np.float32).reshape(CQK, 1), (2, 1))

    # pack all weights into one [128, WPACK_B] u8 blob (single DMA):
    # per partition: bq(4B) bk(4B) wq(512B) wk(512B) wv(2048B)
    wpk = np.concatenate([
        np.ascontiguousarray(bq2).view(np.uint8),
        np.ascontiguousarray(bk2).view(np.uint8),
        np.ascontiguousarray(wqT.reshape(128, CC * 128)).view(np.uint8),
        np.ascontiguousarray(wkT.reshape(128, CC * 128)).view(np.uint8),
        np.ascontiguousarray(wvT.reshape(128, CC * C)).view(np.uint8),
    ], axis=1)
    assert wpk.shape == (128, WPACK_B), wpk.shape

    in_maps = []
    for i in range(N_CORES):
        sl = slice(i * BPC, (i + 1) * BPC)
        in_maps.append({
            "face8": face8[sl], "audio8": audio8[sl], "faceb": faceb[sl],
            "wpack": wpk,
        })
    return in_maps


def kernel(face_feat, audio_feat, Wq, bq, Wk, bk, Wv, bv, gamma):
    nc = _build_program()
    in_maps = _make_in_maps(face_feat, audio_feat, Wq, bq, Wk, bk, Wv, bv,
                            gamma)
    res = run_bass_kernel_spmd(nc, in_maps, core_ids=list(range(N_CORES)))
    out = np.concatenate([res.results[i]["out"] for i in range(N_CORES)],
                         axis=0)
    return out.reshape(B, C, N).astype(np.float32).reshape(B, C, H, W)


# revision 36
# speedup vs baseline: 1.0054x; 1.0054x over previous
"""Trainium2 Bass kernel for CrossModalAttention2d (pipelined rewrite).

Reference computation (per batch element b):
    q = Wq @ face[b] + bq          # [64, 1024]   (face as [C=512, N=1024])
    k = Wk @ audio[b] + bk         # [64, 1024]
    v = Wv @ audio[b] + bv         # [512, 1024]
    attn = softmax(q^T k / 8, axis=-1)          # [1024, 1024]
    out = gamma * (v @ attn^T) + face[b]        # [512, 1024]

Distribution: data-parallel over batch B=32 across 8 NeuronCores
(4 batch elements per core); every core holds the full (small) weights.

Design notes (v2 — software-pipelined, engine-balanced):
- 2-stage pipeline per core: iter n runs {proj q/k/v, energy, exp} for
  batch n on TensorE/ACT while {PV, softmax-normalize, residual, out-DMA}
  for batch n-1 run concurrently.  Tensor issue order interleaves the two
  so the PE never drains a single dependency chain.
- Energy is computed transposed ET[nk,nq] = k^T q with bf16 operands and
  K=128 (q/k host-duplicated into both partition halves => ET = 2E).
- PSUM->SBUF traffic is the scarce resource (GPSIMD has no PSUM access on
  TRN2, so only ACT+DVE can drain PSUM): every such op is [128,1024]
  (PSUM pairs merged) to amortize the ~200-370ns fixed access latency.
  GPSIMD gets the one SBUF-only bulk job: the bf16 residual adds.
- Softmax denominators come from a 5th PV chunk whose stationary is an
  all-ones fp8 tile: S lands broadcast across all 128 partitions in the
  same PSUM rotation the PV chunks use (no separate sum pass).
- PSUM budget = exactly 8 banks: tag "ep" 2x[128,1024] (energy groups) +
  tag "u" 2x[128,2,512] rotating (proj / vproj / PV / sum pairs).
- All heavy matmuls run fp8 DoubleRow (0.5 cyc/row): projections, vproj,
  PV.  Energy is bf16 (the K=64 contraction can't fill 256 rows).
- Weights are host-scaled x16 to move their ~0.02-magnitude entries out
  of fp8-denormal territory; the 1/16 is folded into the cast scales /
  the PV normalize multiply.  gamma and gamma*bv are folded on host into
  Wv and the bf16 residual respectively (exact: attn rows sum to 1).
- Residual path is bf16 end-to-end: face bf16 in, out bf16 back to HBM,
  upcast to fp32 on host (|err| ~ 2^-9 << 2e-2 gate).
- A dozen warm-up matmuls on constant tiles run during the initial DMAs
  so the PE p-state ramp (0.65/1.2 GHz -> 2.4 GHz after ~3us busy) is
  paid before real work; a dummy Exp preloads the ACT table.
"""

from contextlib import ExitStack

import ml_dtypes
import numpy as np

import concourse.bass as bass
import concourse.mybir as mybir
import concourse.tile as tile
from concourse import bacc
from concourse.bass import ds
from concourse.bass_utils import run_bass_kernel_spmd

N_CORES = 8
B = 32
C = 512
CQK = 64
N = 1024          # Nq = Nk = 32*32
H = W = 32
BPC = B // N_CORES  # batches per core
CC = C // 128       # 4 c-chunks
NT = N // 128       # 8 nk-tiles

BF16 = mybir.dt.bfloat16
FP8 = mybir.dt.float8e4
F32 = mybir.dt.float32
DR = mybir.MatmulPerfMode.DoubleRow

WS = 16.0           # host-side weight scale (fp8 denormal avoidance)
IWS = 1.0 / WS      # folded back on device
# energy PSUM = 2*E (q/k partition-duplicated); softmax wants exp(E/8)
EXP_SCALE = 1.0 / 16.0
# packed weights, bytes per partition: bq(4) bk(4) wq(512) wk(512) wv(2048)
WPACK_B = 3080

_PROGRAM = None


def _emit(nc, tc, ctx, io):
    face8, audio8, faceb, wpack, out = io
    AF = mybir.ActivationFunctionType
    ALU = mybir.AluOpType

    wpool = ctx.enter_context(tc.tile_pool(name="w", bufs=1))
    sb = ctx.enter_context(tc.tile_pool(name="sb", bufs=2))
    pp = ctx.enter_context(tc.tile_pool(name="pp", bufs=1, space="PSUM"))

    # --- constants + PE warm-up FIRST, with memsets on GPSIMD (its
    # queue is free earliest), so the HAM clock gate is released while
    # the first input DMAs are still in flight.  Memset through a u32
    # bitcast (4x fewer columns: fp8 memsets run at 1 byte/lane/cycle) ---
    ones = wpool.tile([128, 2, 128], FP8, tag="ones")
    nc.gpsimd.memset(ones[:].bitcast(mybir.dt.uint32), 0x38383838)  # fp8 1.0
    # preload the ACT exp table while DMAs run
    dum_i = wpool.tile([128, 1], F32, tag="dumi")
    nc.gpsimd.memset(dum_i[:], 0.0)
    dum_o = wpool.tile([128, 1], F32, tag="dumo")
    nc.scalar.activation(dum_o[:], dum_i[:], AF.Exp)

    # ~26 x 128-col warm matmuls on `ones` (~3.4us of cold matmuls = one
    # HAM window releases the clock gate); same stationary throughout so
    # LDWEIGHTS runs once
    for i in range(13):
        wp = pp.tile([128, 2, 512], F32, tag="u", bufs=2, name=f"warm{i}")
        nc.tensor.matmul(wp[:, 0, 0:128], ones[:], ones[:], start=True,
                         stop=True, perf_mode=DR)
        nc.tensor.matmul(wp[:, 1, 0:128], ones[:], ones[:], start=True,
                         stop=True, perf_mode=DR)

    # --- all weights in ONE packed DMA (per-DMA queue overhead is
    # ~600ns; five separate weight DMAs would push wv past vproj(0)) ---
    wp_sb = wpool.tile([128, WPACK_B], mybir.dt.uint8, tag="wpack")
    nc.sync.dma_start(wp_sb[:], wpack[:])
    bq_sb = wp_sb[:, 0:4].bitcast(F32)
    bk_sb = wp_sb[:, 4:8].bitcast(F32)
    wq_sb = wp_sb[:, 8:520].bitcast(FP8).rearrange("p (c m) -> p c m", c=CC)
    wk_sb = wp_sb[:, 520:1032].bitcast(FP8).rearrange("p (c m) -> p c m", c=CC)
    wv_sb = wp_sb[:, 1032:3080].bitcast(FP8).rearrange("p (c m) -> p c m", c=CC)

    # --- input DMA helper ---
    ins = {}

    def dma_in(b, split=False):
        f8 = sb.tile([128, CC, N], FP8, tag="face8", name=f"f8_{b}")
        a8 = sb.tile([128, CC, N], FP8, tag="aud8", name=f"a8_{b}")
        fb = sb.tile([128, CC, N], BF16, tag="faceb", bufs=3, name=f"fb_{b}")
        if split:
            # batch 0 rides the startup critical path: split the loads by
            # cc-halves so the first K=256 projection pass starts sooner
            for h in (0, 2):
                nc.sync.dma_start(f8[:, h:h + 2, :],
                                  face8[b, h:h + 2].transpose([1, 0, 2]))
                nc.sync.dma_start(a8[:, h:h + 2, :],
                                  audio8[b, h:h + 2].transpose([1, 0, 2]))
        else:
            nc.sync.dma_start(f8[:], face8[b].transpose([1, 0, 2]))
            nc.sync.dma_start(a8[:], audio8[b].transpose([1, 0, 2]))
        nc.sync.dma_start(fb[:], faceb[b].transpose([1, 0, 2]))
        ins[b] = (f8, a8, fb)

    dma_in(0, split=True)

    prev = None  # state of batch n-1: (b, pt, vt, recip, fb, outb)

    for n in range(BPC + 1):
        if n + 1 < BPC:
            dma_in(n + 1)

        have_f = n < BPC
        have_p = prev is not None

        if have_f:
            f8, a8, fb = ins.pop(n)
            # --- q/k projections (fp8 DoubleRow, K=512 as 2x256) ---
            qp = pp.tile([128, 2, 512], F32, tag="u", bufs=2, name=f"qp{n}")
            for kk in (0, 2):
                for j in range(2):
                    nc.tensor.matmul(qp[:, j, :], wq_sb[:, kk:kk + 2, :],
                                     f8[:, kk:kk + 2, ds(j * 512, 512)],
                                     start=(kk == 0), stop=(kk == 2),
                                     perf_mode=DR)
            q_sb = sb.tile([128, N], BF16, tag="q", name=f"q{n}")
            nc.scalar.activation(q_sb[:], qp[:], AF.Identity,
                                 bias=bq_sb[:], scale=IWS)

            kp = pp.tile([128, 2, 512], F32, tag="u", bufs=2, name=f"kp{n}")
            for kk in (0, 2):
                for j in range(2):
                    nc.tensor.matmul(kp[:, j, :], wk_sb[:, kk:kk + 2, :],
                                     a8[:, kk:kk + 2, ds(j * 512, 512)],
                                     start=(kk == 0), stop=(kk == 2),
                                     perf_mode=DR)
            k_sb = sb.tile([128, N], BF16, tag="k", name=f"k{n}")
            # split j-halves so energy group 0 unblocks after the first
            for j in range(2):
                nc.vector.tensor_scalar(k_sb[:, ds(j * 512, 512)],
                                        kp[:, j, :], IWS, bk_sb[:],
                                        ALU.mult, ALU.add)

            pt_sb = sb.tile([128, NT, N], FP8, tag="pt", name=f"pt{n}")
            vt_sb = sb.tile([128, NT, C], FP8, tag="vt", name=f"vt{n}")

        # --- PV piece emitters (interleaved into g-slots). `st` is the
        # per-batch PV state: [bp, pt, vt, fb, outb, op_s, recip] ---
        def piece_ops(st, tts):
            bp, pt_p, vt_p, fb_p, outb_p = st[:5]
            if st[5] is None:
                st[5] = pp.tile([128, 2, 512], F32, tag="u", bufs=2,
                                name=f"ops{n}_{bp}")
            op_s = st[5]
            for tt in tts:
                for j in range(2):
                    nc.tensor.matmul(op_s[:, j, :], ones[:],
                                     pt_p[:, tt:tt + 2, ds(j * 512, 512)],
                                     start=(tt == 0), stop=(tt == 6),
                                     perf_mode=DR)

        def piece_recip(st):
            recip = sb.tile([128, N], F32, tag="recip", name=f"rc{n}_{st[0]}")
            nc.vector.reciprocal_approx_fast(recip[:], st[5][:])
            st[6] = recip

        def piece_cc(st, cc, drain=False):
            bp, pt_p, vt_p, fb_p, outb_p = st[:5]
            recip = st[6]
            op = pp.tile([128, 2, 512], F32, tag="u", bufs=2,
                         name=f"op{n}_{bp}_{cc}")
            for tt in (0, 2, 4, 6):
                for j in range(2):
                    nc.tensor.matmul(op[:, j, :],
                                     vt_p[:, tt:tt + 2, ds(cc * 128, 128)],
                                     pt_p[:, tt:tt + 2, ds(j * 512, 512)],
                                     start=(tt == 0), stop=(tt == 6),
                                     perf_mode=DR)
            # out_attn = (op * 1/16) * recip   (wv was x16 on host)
            tmp = sb.tile([128, N], BF16, tag="tmp", bufs=3,
                          name=f"tmp{n}_{bp}_{cc}")
            if not drain:
                # steady state: one 1024-col normalize on DVE, add on
                # GPSIMD (SBUF-only bf16)
                nc.vector.scalar_tensor_tensor(tmp[:], op[:], IWS, recip[:],
                                               ALU.mult, ALU.mult)
                nc.gpsimd.tensor_add(outb_p[:, cc, :], tmp[:], fb_p[:, cc, :])
            else:
                # drain iteration: j-split chain fully on DVE so the tail
                # after the last matmul is short (gpsimd adds are 7x slower)
                for j in range(2):
                    js = ds(j * 512, 512)
                    nc.vector.scalar_tensor_tensor(
                        tmp[:, js], op[:, j, :], IWS, recip[:, js],
                        ALU.mult, ALU.mult)
                    nc.vector.tensor_add(outb_p[:, cc, js],
                                         tmp[:, js], fb_p[:, cc, js])
            nc.sync.dma_start(out[bp, cc], outb_p[:, cc, :])

        if have_p and have_f:
            st_p = prev
            if n == 1:
                # fill iteration: exp(0) is still draining on ACT — place
                # PV(0) in the back half so the in-order tensor queue
                # never blocks on a not-yet-exp'd PT tile
                slot_plan = {4: [lambda: piece_ops(st_p, (0, 2))],
                             5: [lambda: piece_ops(st_p, (4, 6)),
                                 lambda: piece_recip(st_p)],
                             6: [lambda: piece_cc(st_p, 0),
                                 lambda: piece_cc(st_p, 1)],
                             7: [lambda: piece_cc(st_p, 2),
                                 lambda: piece_cc(st_p, 3)]}
            else:
                slot_plan = {0: [lambda: piece_ops(st_p, (0, 2))],
                             1: [lambda: piece_ops(st_p, (4, 6)),
                                 lambda: piece_recip(st_p)],
                             2: [lambda: piece_cc(st_p, 0)],
                             3: [lambda: piece_cc(st_p, 1)],
                             4: [lambda: piece_cc(st_p, 2)],
                             5: [lambda: piece_cc(st_p, 3)]}
        else:
            slot_plan = {}

        # --- interleaved: energy+exp+vproj (batch n) / PV chunks (n-1) ---
        for g in range(NT):
            if have_f:
                # energy group g: ET[g-tile, :] = k^T q  (bf16, K=128 dup)
                ep = pp.tile([128, 1024], F32, tag="ep", bufs=2,
                             name=f"ep{n}_{g}")
                for j in range(2):
                    nc.tensor.matmul(ep[:, ds(j * 512, 512)],
                                     k_sb[:, ds(g * 128, 128)],
                                     q_sb[:, ds(j * 512, 512)],
                                     start=True, stop=True)
                nc.scalar.activation(pt_sb[:, g, :], ep[:], AF.Exp,
                                     scale=EXP_SCALE)
                # vproj pair: Vt[2 nk-tiles, c] = (16*gamma*Wv) @ audio
                if g % 2 == 0:
                    vp = pp.tile([128, 2, 512], F32, tag="u", bufs=2,
                                 name=f"vp{n}_{g}")
                for kk in (0, 2):
                    nc.tensor.matmul(vp[:, g % 2, :],
                                     a8[:, kk:kk + 2, ds(g * 128, 128)],
                                     wv_sb[:, kk:kk + 2, :],
                                     start=(kk == 0), stop=(kk == 2),
                                     perf_mode=DR)
                if g % 2 == 1:
                    # vt cast: 1 of 4 on ACT (its only slack), rest on DVE
                    if g == 1:
                        nc.scalar.activation(vt_sb[:, g - 1:g + 1, :], vp[:],
                                             AF.Copy)
                    else:
                        nc.vector.tensor_copy(vt_sb[:, g - 1:g + 1, :], vp[:])
            for p in slot_plan.get(g, []):
                p()

        if have_p and not have_f:
            # drain iteration: no forward work to interleave with
            piece_ops(prev, (0, 2))
            piece_ops(prev, (4, 6))
            piece_recip(prev)
            for cc in range(CC):
                piece_cc(prev, cc, drain=True)

        if have_f:
            outb = sb.tile([128, CC, N], BF16, tag="outb", name=f"ob{n}")
            prev = [n, pt_sb, vt_sb, fb, outb, None, None]
        else:
            prev = None


def _build_program():
    global _PROGRAM
    if _PROGRAM is not None:
        return _PROGRAM
    nc = bacc.Bacc("TRN2", target_bir_lowering=False, debug=False,
                   num_devices=N_CORES)
    d = {}
    d["face8"] = nc.dram_tensor("face8", [BPC, CC, 128, N], FP8,
                                kind="ExternalInput").ap()
    d["audio8"] = nc.dram_tensor("audio8", [BPC, CC, 128, N], FP8,
                                 kind="ExternalInput").ap()
    d["faceb"] = nc.dram_tensor("faceb", [BPC, CC, 128, N], BF16,
                                kind="ExternalInput").ap()
    d["wpack"] = nc.dram_tensor("wpack", [128, WPACK_B], mybir.dt.uint8,
                                kind="ExternalInput").ap()
    d["out"] = nc.dram_tensor("out", [BPC, CC, 128, N], BF16,
                              kind="ExternalOutput").ap()

    io = (d["face8"], d["audio8"], d["faceb"], d["wpack"], d["out"])
    with tile.TileContext(nc) as tc:
        with ExitStack() as ctx:
            _emit(nc, tc, ctx, io)
    nc.compile()
    _PROGRAM = nc
    return nc


def _make_in_maps(face_feat, audio_feat, Wq, bq, Wk, bk, Wv, bv, gamma):
    bf16 = ml_dtypes.bfloat16
    fp8 = ml_dtypes.float8_e4m3fn
    face = np.ascontiguousarray(face_feat.reshape(B, C, N), dtype=np.float32)
    audio = np.ascontiguousarray(audio_feat.reshape(B, C, N), dtype=np.float32)
    g = np.float32(np.asarray(gamma).reshape(-1)[0])

    # residual folds in gamma*bv (v-bias passes through softmax exactly)
    faceb = (face + (g * bv.astype(np.float32))[None, :, None])
    faceb = faceb.astype(bf16).reshape(B, CC, 128, N)

    face8 = face.astype(fp8).reshape(B, CC, 128, N)
    audio8 = audio.astype(fp8).reshape(B, CC, 128, N)

    def chunk_t(wT):  # [C, M] -> [128, CC, M]
        return np.ascontiguousarray(
            wT.reshape(CC, 128, -1).transpose(1, 0, 2))

    # q/k weights duplicated along M so the projections emit q/k into both
    # partition halves (energy contracts K=128 = 2x the true 64).
    # All weights are scaled x16 on host to stay clear of fp8 denormals;
    # the device folds the 1/16 back in (cast scales / normalize mult).
    wqT = chunk_t((np.concatenate([Wq.T, Wq.T], axis=1)
                   .astype(np.float32) * WS).astype(fp8))
    wkT = chunk_t((np.concatenate([Wk.T, Wk.T], axis=1)
                   .astype(np.float32) * WS).astype(fp8))
    wvT = chunk_t((Wv.astype(np.float32).T * (WS * g)).astype(fp8))
    bq2 = np.tile(bq.astype(np.float32).reshape(CQK, 1), (2, 1))
    bk2 = np.tile(bk.astype(np.float32).reshape(CQK, 1), (2, 1))

    # pack all weights into one [128, WPACK_B] u8 blob (single DMA):
    # per partition: bq(4B) bk(4B) wq(512B) wk(512B) wv(2048B)
    wpk = np.concatenate([
        np.ascontiguousarray(bq2).view(np.uint8),
        np.ascontiguousarray(bk2).view(np.uint8),
        np.ascontiguousarray(wqT.reshape(128, CC * 128)).view(np.uint8),
        np.ascontiguousarray(wkT.reshape(128, CC * 128)).view(np.uint8),
        np.ascontiguousarray(wvT.reshape(128, CC * C)).view(np.uint8),
    ], axis=1)
    assert wpk.shape == (128, WPACK_B), wpk.shape

    in_maps = []
    for i in range(N_CORES):
        sl = slice(i * BPC, (i + 1) * BPC)
        in_maps.append({
            "face8": face8[sl], "audio8": audio8[sl], "faceb": faceb[sl],
            "wpack": wpk,
        })
    return in_maps


def kernel(face_feat, audio_feat, Wq, bq, Wk, bk, Wv, bv, gamma):
    nc = _build_program()
    in_maps = _make_in_maps(face_feat, audio_feat, Wq, bq, Wk, bk, Wv, bv,
                            gamma)
    res = run_bass_kernel_spmd(nc, in_maps, core_ids=list(range(N_CORES)))
    out = np.concatenate([res.results[i]["out"] for i in range(N_CORES)],
                         axis=0)
    return out.reshape(B, C, N).astype(np.float32).reshape(B, C, H, W)
